# revision 6
# baseline (speedup 1.0000x reference)
"""Trainium2 Bass kernel for nn_Cluster_loss (segment_reduce).

Strategy (data-parallel over batch N=8, one image per NeuronCore):
  Per image f [C=32, P=262144] fp32, gt [P] int32 with K=16 clusters.

  Pixel tiling: p = s*2048 + b*512 + q*128 + j   (s: supergroup 0..127,
  b: block 0..3, q: chunk 0..3, j: 0..127).

  Pass 1 (per supergroup):
    - load f_stacked [(c,b)=128, 512] (natural rows, 2KB contiguous runs)
    - PE transpose per chunk -> fT [j=128, (c,b)=128] in PSUM
    - ACT copies fT->SBUF and fT^2->SBUF; DVE reduces over c -> s2 [j,(q,b)]
    - one-hot OH [j, (q,b,k)] built on GPSIMD from host-transposed gtT
    - PE segment-sum matmul: OH^T @ fT accumulated over all pixels
      -> sums[(b,k),(c,b')] in PSUM (block-diagonal b=b' wanted)
  Finalize on device: sums -> means (via host-provided 1/counts), R =
  -2*mu scatter, m2 = ||mu||^2 row.
  Pass 2 (per supergroup):
    - reload f_stacked; PE: g2m[j,(q,b,k)] = m2[k] - 2*mu[.,k].f_pixel
      (rank-1 m2 pre-seed matmul + 4 G-matmuls accumulating into PSUM)
    - DVE: w = sum_k OH*g2m ; dist2 = s2 + w ; clamp; ACT sqrt -> dist
    - PE segment-sum of dist -> segdist[(b'),(b,k)] accumulated in PSUM
  Host: counts via bincount, losses assembled in float64, exact reference
  formula; returns (total, (var_loss, dist_loss, norm_loss, means)).
"""

import numpy as np

import concourse.bacc as bacc
import concourse.bass as bass
import concourse.mybir as mybir
from concourse import tile
from concourse.bass_utils import run_bass_kernel_spmd

# Problem constants (hardcoded per contract)
N, C, H, W = 8, 32, 512, 512
HW = H * W  # 262144
K = 16
DELTA_V = 0.2
DELTA_D = 0.2
ALPHA, BETA, GAMMA = 1.0, 1.0, 0.001

NSG = 128          # supergroups per image (2048 px each)
SGPX = 2048
NB = 4             # blocks per supergroup
NQ = 4             # chunks per supergroup
GB = 16            # supergroups per DMA batch

F32 = mybir.dt.float32


def _build_nc(nsg: int):
    nc = bacc.Bacc("TRN2", target_bir_lowering=False, debug=False)
    hw = nsg * SGPX

    f = nc.dram_tensor("f", [C, hw], F32, kind="ExternalInput")
    gtT = nc.dram_tensor("gtT", [128, nsg * 16], F32, kind="ExternalInput")
    i128 = nc.dram_tensor("i128", [128, 128], F32, kind="ExternalInput")
    iota16 = nc.dram_tensor("iota16", [128, 16], F32, kind="ExternalInput")
    ones_row = nc.dram_tensor("ones_row", [1, 128], F32, kind="ExternalInput")
    recip = nc.dram_tensor("recip", [16, 1], F32, kind="ExternalInput")

    sums_o = nc.dram_tensor("sums_o", [16, C], F32, kind="ExternalOutput")
    segdist_o = nc.dram_tensor("segdist_o", [4, 64], F32, kind="ExternalOutput")

    nbatch = nsg // GB

    with tile.TileContext(nc) as tc:
        with (
            tc.tile_pool(name="persist", bufs=1) as pp,
            tc.tile_pool(name="psum_persist", bufs=1, space="PSUM") as ppsum,
        ):
            # Resident tiles
            gtT_sb = pp.tile([128, nsg * 16], F32)
            nc.sync.dma_start(out=gtT_sb[:], in_=gtT[:])
            s2_all = pp.tile([128, nsg * 16], F32)
            i128_sb = pp.tile([128, 128], F32)
            nc.sync.dma_start(out=i128_sb[:], in_=i128[:])
            iota16_sb = pp.tile([128, 16], F32)
            nc.sync.dma_start(out=iota16_sb[:], in_=iota16[:])
            ones_row_sb = pp.tile([1, 128], F32)
            nc.sync.dma_start(out=ones_row_sb[:], in_=ones_row[:])
            recip_sb = pp.tile([16, 1], F32)
            nc.sync.dma_start(out=recip_sb[:], in_=recip[:])

            sums_ps = ppsum.tile([64, 128], F32, space="PSUM")
            segdist_ps = ppsum.tile([4, 64], F32, space="PSUM")

            # ---------------- PASS 1 ----------------
            with (
                tc.tile_pool(name="fstk", bufs=2) as fstk_pool,
                tc.tile_pool(name="ft", bufs=2) as ft_pool,
                tc.tile_pool(name="oh1", bufs=2) as oh1_pool,
                tc.tile_pool(name="ps1", bufs=2, space="PSUM") as ps1_pool,
            ):
                for ib in range(nbatch):
                    fstk = fstk_pool.tile([128, GB * 512], F32, tag="fstk")
                    srcv = f[:, ib * GB * SGPX:(ib + 1) * GB * SGPX].rearrange(
                        "c (g b j2) -> c b g j2", g=GB, b=NB
                    )
                    for b in range(NB):
                        nc.sync.dma_start(
                            out=fstk[b * C:(b + 1) * C, :].rearrange(
                                "p (g j2) -> p g j2", g=GB),
                            in_=srcv[:, b, :, :])
                    for g in range(GB):
                        s = ib * GB + g
                        ft_ps = ps1_pool.tile([128, 512], F32, space="PSUM",
                                              tag="ftps")
                        for q in range(NQ):
                            nc.tensor.transpose(
                                out=ft_ps[:, q * 128:(q + 1) * 128],
                                in_=fstk[:, g * 512 + q * 128:g * 512 + (q + 1) * 128],
                                identity=i128_sb[:],
                            )
                        ft_sb = ft_pool.tile([128, 512], F32, tag="ftsb")
                        nc.scalar.copy(out=ft_sb[:], in_=ft_ps[:])
                        fsq = ft_pool.tile([128, 512], F32, tag="fsq")
                        nc.scalar.square(out=fsq[:], in_=ft_ps[:])
                        # s2[j, (q,b)] = sum_c fsq ; col = q*128 + c*4 + b
                        nc.vector.tensor_reduce(
                            out=s2_all[:, s * 16:(s + 1) * 16],
                            in_=fsq[:].rearrange("j (q b c) -> j (q b) c",
                                                 q=NQ, b=NB, c=C),
                            op=mybir.AluOpType.add,
                            axis=mybir.AxisListType.X,
                        )
                        # one-hot [j, (q, b, k)]
                        oh = oh1_pool.tile([128, 256], F32, tag="oh")
                        gts = gtT_sb[:, s * 16:(s + 1) * 16].rearrange(
                            "j (q b) -> j q b", q=NQ
                        ).unsqueeze(3).broadcast_to([128, NQ, NB, K])
                        iot = iota16_sb[:].unsqueeze(1).unsqueeze(1).broadcast_to(
                            [128, NQ, NB, K]
                        )
                        nc.vector.tensor_tensor(
                            out=oh[:].rearrange("j (q b k) -> j q b k", q=NQ, b=NB),
                            in0=gts, in1=iot, op=mybir.AluOpType.is_equal,
                        )
                        for q in range(NQ):
                            nc.tensor.matmul(
                                out=sums_ps[:],
                                lhsT=oh[:, q * 64:(q + 1) * 64],
                                rhs=ft_sb[:, q * 128:(q + 1) * 128],
                                start=(s == 0 and q == 0),
                                stop=(s == nsg - 1 and q == NQ - 1),
                            )

            # ---------------- FINALIZE (means, R, m2) ----------------
            with (
                tc.tile_pool(name="fin", bufs=1) as fin,
                tc.tile_pool(name="finps", bufs=1, space="PSUM") as finps,
            ):
                sums_all = fin.tile([64, 128], F32)
                nc.vector.tensor_copy(out=sums_all[:], in_=sums_ps[:])
                # gather diag blocks sums_all[b*16+k, b*32+c] via tiny DMAs
                # into diag [16, (c,b)] then reduce over b
                diag = fin.tile([16, 128], F32)
                dv = diag[:].rearrange("k (c b) -> k b c", b=NB)
                for b in range(NB):
                    nc.sync.dma_start(
                        out=dv[:, b, :],
                        in_=sums_all[b * 16:(b + 1) * 16, b * C:(b + 1) * C],
                    )
                sums_sb = fin.tile([16, C], F32)
                nc.vector.tensor_reduce(
                    out=sums_sb[:],
                    in_=diag[:].rearrange("k (c b) -> k c b", b=NB),
                    op=mybir.AluOpType.add, axis=mybir.AxisListType.X,
                )
                nc.sync.dma_start(out=sums_o[:], in_=sums_sb[:])

                muT = fin.tile([16, C], F32)  # mu[k, c]
                nc.vector.tensor_scalar(
                    out=muT[:], in0=sums_sb[:], scalar1=recip_sb[:, 0:1],
                    scalar2=None, op0=mybir.AluOpType.mult,
                )
                # mu [c, k] via PE transpose
                mu_ps = finps.tile([C, 16], F32, space="PSUM")
                nc.tensor.transpose(out=mu_ps[:], in_=muT[:],
                                    identity=i128_sb[0:16, 0:16])
                mu_sb = fin.tile([C, 16], F32)
                nc.scalar.copy(out=mu_sb[:], in_=mu_ps[:])
                # R[(b,c), (b,k)] = -2*mu[c,k] on the block diagonal
                r_sb = fin.tile([128, 64], F32)
                nc.vector.memset(r_sb[:], 0.0)
                for b in range(NB):
                    nc.vector.tensor_scalar(
                        out=r_sb[b * C:(b + 1) * C, b * 16:(b + 1) * 16],
                        in0=mu_sb[:], scalar1=-2.0, scalar2=None,
                        op0=mybir.AluOpType.mult,
                    )

                # m2[k] = sum_c mu^2 -> row [1, 256] over (q, b, k)
                mu2 = fin.tile([16, C], F32)
                nc.vector.tensor_tensor(out=mu2[:], in0=muT[:], in1=muT[:],
                                        op=mybir.AluOpType.mult)
                m2c = fin.tile([16, 1], F32)
                nc.vector.tensor_reduce(out=m2c[:], in_=mu2[:],
                                        op=mybir.AluOpType.add,
                                        axis=mybir.AxisListType.X)
                m2r_ps = finps.tile([1, 16], F32, space="PSUM")
                nc.tensor.transpose(out=m2r_ps[:], in_=m2c[:],
                                    identity=i128_sb[0:16, 0:16])
                m2r = fin.tile([1, 16], F32)
                nc.scalar.copy(out=m2r[:], in_=m2r_ps[:])
                m2row = fin.tile([1, 256], F32)
                nc.vector.tensor_copy(
                    out=m2row[:].rearrange("o (q b k) -> o q b k", q=NQ, b=NB),
                    in_=m2r[:].unsqueeze(1).unsqueeze(1).broadcast_to(
                        [1, NQ, NB, K]),
                )

                # ---------------- PASS 2 ----------------
                with (
                    tc.tile_pool(name="fstk2", bufs=2) as fstk2_pool,
                    tc.tile_pool(name="w2", bufs=2) as w2_pool,
                    tc.tile_pool(name="ps2", bufs=2, space="PSUM") as ps2_pool,
                ):
                    for ib in range(nbatch):
                        fstk = fstk2_pool.tile([128, GB * 512], F32, tag="fstk2")
                        srcv = f[:, ib * GB * SGPX:(ib + 1) * GB * SGPX].rearrange(
                            "c (g b j2) -> c b g j2", g=GB, b=NB
                        )
                        for b in range(NB):
                            nc.sync.dma_start(
                                out=fstk[b * C:(b + 1) * C, :].rearrange(
                                    "p (g j2) -> p g j2", g=GB),
                                in_=srcv[:, b, :, :])
                        for g in range(GB):
                            s = ib * GB + g
                            g2m = ps2_pool.tile([128, 256], F32, space="PSUM",
                                                tag="g2m")
                            nc.tensor.matmul(
                                out=g2m[:], lhsT=ones_row_sb[:], rhs=m2row[:],
                                start=True, stop=False,
                            )
                            for q in range(NQ):
                                nc.tensor.matmul(
                                    out=g2m[:, q * 64:(q + 1) * 64],
                                    lhsT=fstk[:, g * 512 + q * 128:
                                              g * 512 + (q + 1) * 128],
                                    rhs=r_sb[:],
                                    start=False, stop=(q == NQ - 1),
                                )
                            oh = w2_pool.tile([128, 256], F32, tag="oh2")
                            gts = gtT_sb[:, s * 16:(s + 1) * 16].rearrange(
                                "j (q b) -> j q b", q=NQ
                            ).unsqueeze(3).broadcast_to([128, NQ, NB, K])
                            iot = iota16_sb[:].unsqueeze(1).unsqueeze(1)\
                                .broadcast_to([128, NQ, NB, K])
                            nc.vector.tensor_tensor(
                                out=oh[:].rearrange("j (q b k) -> j q b k",
                                                    q=NQ, b=NB),
                                in0=gts, in1=iot, op=mybir.AluOpType.is_equal,
                            )
                            v = w2_pool.tile([128, 256], F32, tag="v")
                            nc.vector.tensor_tensor(
                                out=v[:], in0=oh[:], in1=g2m[:],
                                op=mybir.AluOpType.mult,
                            )
                            w = w2_pool.tile([128, 16], F32, tag="w")
                            nc.vector.tensor_reduce(
                                out=w[:],
                                in_=v[:].rearrange("j (qb k) -> j qb k", k=K),
                                op=mybir.AluOpType.add,
                                axis=mybir.AxisListType.X,
                            )
                            d2 = w2_pool.tile([128, 16], F32, tag="d2")
                            nc.vector.tensor_tensor(
                                out=d2[:], in0=w[:],
                                in1=s2_all[:, s * 16:(s + 1) * 16],
                                op=mybir.AluOpType.add,
                            )
                            d2c = w2_pool.tile([128, 16], F32, tag="d2c")
                            nc.vector.tensor_scalar(
                                out=d2c[:], in0=d2[:], scalar1=0.0, scalar2=None,
                                op0=mybir.AluOpType.max,
                            )
                            dist = w2_pool.tile([128, 16], F32, tag="dist")
                            nc.scalar.sqrt(out=dist[:], in_=d2c[:])
                            for q in range(NQ):
                                nc.tensor.matmul(
                                    out=segdist_ps[:],
                                    lhsT=dist[:, q * 4:(q + 1) * 4],
                                    rhs=oh[:, q * 64:(q + 1) * 64],
                                    start=(s == 0 and q == 0),
                                    stop=(s == nsg - 1 and q == NQ - 1),
                                )

                segd_sb = fin.tile([4, 64], F32)
                nc.vector.tensor_copy(out=segd_sb[:], in_=segdist_ps[:])
                nc.sync.dma_start(out=segdist_o[:], in_=segd_sb[:])

    nc.finalize()
    return nc


_NC_CACHE = {}


def _get_nc(nsg: int):
    if nsg not in _NC_CACHE:
        _NC_CACHE[nsg] = _build_nc(nsg)
    return _NC_CACHE[nsg]


def make_in_maps(features: np.ndarray, ground_truth: np.ndarray, nsg: int = NSG):
    """Build per-core input maps + host-side per-image count info."""
    n, c, h, w = features.shape
    hw = h * w
    assert hw == nsg * SGPX
    f_flat = features.reshape(n, c, hw)
    gt_flat = ground_truth.reshape(n, hw)

    i128 = np.eye(128, dtype=np.float32)
    iota16 = np.broadcast_to(np.arange(K, dtype=np.float32), (128, K)).copy()
    ones_row = np.ones((1, 128), dtype=np.float32)

    in_maps = []
    counts_all = []
    for i in range(n):
        gt = gt_flat[i]
        counts = np.bincount(gt, minlength=K).astype(np.float64)
        counts_all.append(counts)
        # gtT[j, s*16 + q*4 + b] = gt[s*2048 + b*512 + q*128 + j]
        gmat = gt.reshape(nsg, NB, NQ, 128)
        gtT = np.ascontiguousarray(
            gmat.transpose(3, 0, 2, 1).reshape(128, nsg * 16)
        ).astype(np.float32)
        recip = (1.0 / counts).astype(np.float32).reshape(16, 1)
        in_maps.append({
            "f": np.ascontiguousarray(f_flat[i]),
            "gtT": gtT,
            "i128": i128,
            "iota16": iota16,
            "ones_row": ones_row,
            "recip": recip,
        })
    return in_maps, counts_all


def postprocess(results, counts_all, n):
    """Host finish: exact reference formulas from per-image sums/segdist."""
    means_all = np.zeros((n, C, K), dtype=np.float32)
    var_n = np.zeros(n, dtype=np.float64)
    dloss_acc = 0.0
    nloss_acc = 0.0
    for i in range(n):
        counts = counts_all[i]                      # [K] float64
        sums = results[i]["sums_o"].astype(np.float64)   # [16, C]
        segd = results[i]["segdist_o"].astype(np.float64)  # [4, 64]
        means_kc = sums / counts[:, None]           # [K, C]
        means_all[i] = means_kc.T.astype(np.float32)
        seg_dist = np.zeros(K, dtype=np.float64)
        for b in range(NB):
            seg_dist += segd[b, b * 16:(b + 1) * 16]
        mean_dist = seg_dist / counts
        var_nk = np.maximum(mean_dist - DELTA_V, 0.0) / counts
        var_n[i] = var_nk.sum() / (1.0 / counts).sum()

        diff = means_kc[:, None, :] - means_kc[None, :, :]   # [K, K, C]
        pd = np.linalg.norm(diff, axis=2)
        mean_other = pd.sum(axis=1) / (K - 1)
        dloss_acc += np.maximum(2.0 * DELTA_D - mean_other, 0.0).sum()
        nloss_acc += np.linalg.norm(means_kc, axis=1).sum()

    variance_loss = var_n.mean()
    distance_loss = dloss_acc / (n * K)
    normalization_loss = nloss_acc / (K * n)
    total = ALPHA * variance_loss + BETA * distance_loss + GAMMA * normalization_loss
    f32 = lambda x: np.float32(x)
    return (f32(total), (f32(variance_loss), f32(distance_loss),
                         f32(normalization_loss), means_all))


def kernel(features: np.ndarray, ground_truth: np.ndarray):
    features = np.asarray(features)
    ground_truth = np.asarray(ground_truth)
    n = features.shape[0]
    nc = _get_nc(NSG)
    in_maps, counts_all = make_in_maps(features, ground_truth)
    res = run_bass_kernel_spmd(nc, in_maps, list(range(n)))
    return postprocess(res.results, counts_all, n)


if __name__ == "__main__":
    from reference import setup_inputs  # only when run manually in problem dir
    inputs = {k: np.asarray(v) for k, v in setup_inputs().items()}
    out = kernel(**inputs)
    print("total:", out[0])
